# revision 2
# baseline (speedup 1.0000x reference)
"""Trainium2 Bass kernel for the DifferentiableMemory scatter_memory problem.

Data-parallel over 8 NeuronCores: batch B=32768 is sharded into 8 x 4096 rows.
Host side does layout only (transpose/cast/concat/weight repack); all NN math
(encoder MLP, cosine sims, top-k, importance net) runs on device in bf16 with
fp32 PSUM accumulation.

Device dataflow (per core, 8 superblocks of 512 batch columns):
  activations live transposed [feature, batch]:
    xT        [128, 8, 512]  combined.T chunks (cue | istate | reward/ts/emo | pad)
    h1T       = gelu(W1.T @ xT + b1)        -> [256, 512] bf16
    encT      = W2.T @ h1T + b2             -> [128, 512] bf16
    ssq[b]    = ones.T @ (encT^2)           -> per-batch ||enc||^2 via PE
    sims[b,n] = (encT_q).T @ centT_scaled   -> [128, 500] fp32 (centT pre-divided
                 by ||c||; divide by ||enc|| AFTER top-8: positive per-row scale
                 preserves order). eps-clamp of the reference never binds here
                 (||enc||*||c|| >> 1e-8).
    top8      = nc.vector.max (one DVE instruction, sorted desc) -> take 5
    impT      = sigmoid(w2i.T @ gelu(W1i.T @ xT + b1i) + b2i) * mean(emo)
"""

import numpy as np
import ml_dtypes

BF16 = ml_dtypes.bfloat16

N_CORES = 8
B = 32768
BL = B // N_CORES          # 4096 rows per core
SB = 512                   # superblock: batch columns per iteration
NSB = BL // SB             # 8 superblocks
Q = SB // 128              # 4 x 128-row tiles per superblock
D = 768
H1 = 256
E = 128
N = 500
K = 5
TOT = 902
TOTP = 1024                # padded combined dim: 8 chunks of 128
NCH = TOTP // 128          # 8
DCH = D // 128             # 6

_CACHE = {}


def _build_nc():
    import concourse.bacc as bacc
    import concourse.bass as bass
    import concourse.tile as tile
    from concourse import mybir

    f32 = mybir.dt.float32
    bf16 = mybir.dt.bfloat16
    AF = mybir.ActivationFunctionType
    ts = bass.ts

    nc = bacc.Bacc(None, target_bir_lowering=False)

    combT = nc.dram_tensor("combT", [TOTP, BL], bf16, kind="ExternalInput")
    emo = nc.dram_tensor("emo", [BL, 4], f32, kind="ExternalInput")
    w1 = nc.dram_tensor("w1", [128, DCH, H1], bf16, kind="ExternalInput")
    w2 = nc.dram_tensor("w2", [128, 2, E], bf16, kind="ExternalInput")
    iw1 = nc.dram_tensor("iw1", [128, NCH, 64], bf16, kind="ExternalInput")
    iw2 = nc.dram_tensor("iw2", [64, 1], bf16, kind="ExternalInput")
    b1 = nc.dram_tensor("b1", [128, 2], f32, kind="ExternalInput")
    b2 = nc.dram_tensor("b2", [128, 1], f32, kind="ExternalInput")
    ib1 = nc.dram_tensor("ib1", [64, 1], f32, kind="ExternalInput")
    ib2 = nc.dram_tensor("ib2", [128, 1], f32, kind="ExternalInput")
    centT = nc.dram_tensor("centT", [128, N], bf16, kind="ExternalInput")
    out = nc.dram_tensor("out", [BL, K + 1], f32, kind="ExternalOutput")

    combT_r = combT.rearrange("(c p) b -> p c b", p=128)     # [128, 8, BL]
    emo_r = emo.rearrange("(x p) e -> p x e", p=128)         # [128, 32, 4]
    out_r = out.rearrange("(x p) j -> p x j", p=128)         # [128, 32, 6]

    with tile.TileContext(nc) as tc:
        with (
            tc.tile_pool(name="const", bufs=1) as cpool,
            tc.tile_pool(name="work", bufs=2) as wpool,
            tc.tile_pool(name="simsp", bufs=4) as spool,
            tc.tile_pool(name="small", bufs=2) as opool,
            tc.tile_pool(name="psA", bufs=3, space="PSUM") as psA,
            tc.tile_pool(name="psS", bufs=3, space="PSUM") as psS,
            tc.tile_pool(name="psT", bufs=1, space="PSUM") as psT,
        ):
            # ---- constants / weights (loaded once) ----
            w1t = cpool.tile([128, DCH, H1], bf16)
            nc.sync.dma_start(w1t[:], w1[:])
            w2t = cpool.tile([128, 2, E], bf16)
            nc.sync.dma_start(w2t[:], w2[:])
            iw1t = cpool.tile([128, NCH, 64], bf16)
            nc.sync.dma_start(iw1t[:], iw1[:])
            iw2t = cpool.tile([64, 1], bf16)
            nc.sync.dma_start(iw2t[:], iw2[:])
            b1t = cpool.tile([128, 2], f32)
            nc.sync.dma_start(b1t[:], b1[:])
            b2t = cpool.tile([128, 1], f32)
            nc.sync.dma_start(b2t[:], b2[:])
            ib1t = cpool.tile([64, 1], f32)
            nc.sync.dma_start(ib1t[:], ib1[:])
            ib2t = cpool.tile([128, 1], f32)
            nc.sync.dma_start(ib2t[:], ib2[:])
            centTt = cpool.tile([128, N], bf16)
            nc.sync.dma_start(centTt[:], centT[:])
            onesE = cpool.tile([128, 1], bf16)
            nc.vector.memset(onesE[:], 1.0)

            for sb in range(NSB):
                # ---- load inputs for this superblock ----
                xt = wpool.tile([128, NCH, SB], bf16, tag="xt")
                nc.sync.dma_start(xt[:], combT_r[:, :, ts(sb, SB)])
                emot = wpool.tile([128, Q, 4], f32, tag="emot")
                nc.sync.dma_start(emot[:], emo_r[:, ts(sb, Q), :])

                # ---- encoder layer 1: h1T = gelu(W1.T @ xT + b1) ----
                h1 = wpool.tile([128, 2, SB], bf16, tag="h1")
                for half in range(2):
                    ps = psA.tile([128, SB], f32, tag="mm")
                    for c in range(DCH):
                        nc.tensor.matmul(
                            ps[:],
                            lhsT=w1t[:, c, ts(half, 128)],
                            rhs=xt[:, c, :],
                            start=(c == 0),
                            stop=(c == DCH - 1),
                        )
                    nc.scalar.activation(
                        h1[:, half, :], ps[:], AF.Gelu, bias=b1t[:, half : half + 1]
                    )

                # ---- encoder layer 2: encT = W2.T @ h1T + b2 ----
                ps_enc = psA.tile([128, SB], f32, tag="mm")
                for c in range(2):
                    nc.tensor.matmul(
                        ps_enc[:],
                        lhsT=w2t[:, c, :],
                        rhs=h1[:, c, :],
                        start=(c == 0),
                        stop=(c == 1),
                    )
                encb = wpool.tile([128, SB], bf16, tag="encb")
                nc.scalar.activation(encb[:], ps_enc[:], AF.Identity, bias=b2t[:])
                enc2 = wpool.tile([128, SB], bf16, tag="enc2")
                nc.scalar.activation(enc2[:], encb[:], AF.Square)

                # ---- ||enc||^2 per batch col via PE (partition reduction) ----
                ps_ssq = psT.tile([128, Q], f32, tag="tiny")
                for q in range(Q):
                    nc.tensor.matmul(
                        ps_ssq[:, q : q + 1],
                        lhsT=enc2[:, ts(q, 128)],
                        rhs=onesE[:],
                        start=True,
                        stop=True,
                    )
                e_n = opool.tile([128, Q], f32, tag="e_n")
                nc.scalar.activation(e_n[:], ps_ssq[:], AF.Sqrt)
                rinv = opool.tile([128, Q], f32, tag="rinv")
                nc.vector.reciprocal(rinv[:], e_n[:])

                # ---- sims + top8 per 128-row tile ----
                top8 = opool.tile([128, Q, 8], f32, tag="top8")
                for q in range(Q):
                    ps_sims = psS.tile([128, N], f32, tag="sims")
                    nc.tensor.matmul(
                        ps_sims[:],
                        lhsT=encb[:, ts(q, 128)],
                        rhs=centTt[:],
                        start=True,
                        stop=True,
                    )
                    sims_sb = spool.tile([128, N], f32, tag="sims_sb")
                    if q % 2 == 0:
                        nc.scalar.copy(sims_sb[:], ps_sims[:])
                    else:
                        nc.vector.tensor_copy(sims_sb[:], ps_sims[:])
                    nc.vector.max(top8[:, q, :], sims_sb[:])

                # ---- importance net ----
                ps_imp = psA.tile([64, SB], f32, tag="mm")
                for c in range(NCH):
                    nc.tensor.matmul(
                        ps_imp[:],
                        lhsT=iw1t[:, c, :],
                        rhs=xt[:, c, :],
                        start=(c == 0),
                        stop=(c == NCH - 1),
                    )
                himp = wpool.tile([64, SB], bf16, tag="himp")
                nc.scalar.activation(himp[:], ps_imp[:], AF.Gelu, bias=ib1t[:])

                ps_ic = psT.tile([128, Q], f32, tag="tiny")
                for q in range(Q):
                    nc.tensor.matmul(
                        ps_ic[:, q : q + 1],
                        lhsT=himp[:, ts(q, 128)],
                        rhs=iw2t[:],
                        start=True,
                        stop=True,
                    )
                sg = opool.tile([128, Q], f32, tag="sg")
                nc.scalar.activation(sg[:], ps_ic[:], AF.Sigmoid, bias=ib2t[:])
                esum = opool.tile([128, Q], f32, tag="esum")
                nc.vector.reduce_sum(
                    esum[:], emot[:], axis=mybir.AxisListType.X
                )

                # ---- assemble output tile [128, Q, 6] ----
                ot = opool.tile([128, Q, K + 1], f32, tag="ot")
                for q in range(Q):
                    nc.vector.tensor_scalar_mul(
                        ot[:, q, 0:K], top8[:, q, 0:K], rinv[:, q : q + 1]
                    )
                # imp = sigmoid * 0.25 * sum(emo)
                nc.vector.scalar_tensor_tensor(
                    ot[:, :, K],
                    in0=sg[:],
                    scalar=0.25,
                    in1=esum[:],
                    op0=mybir.AluOpType.mult,
                    op1=mybir.AluOpType.mult,
                )
                nc.sync.dma_start(out_r[:, ts(sb, Q), :], ot[:])

    nc.compile()
    return nc


def _prep_inputs(cue, internal_state, reward, timestamp, emotional_state,
                 centroids, enc_w1, enc_b1, enc_w2, enc_b2,
                 imp_w1, imp_b1, imp_w2, imp_b2):
    f32 = np.float32

    comb = np.zeros((B, TOTP), dtype=f32)
    comb[:, :D] = cue
    comb[:, D : D + E] = internal_state
    comb[:, D + E] = reward[:, 0]
    comb[:, D + E + 1] = timestamp[:, 0]
    comb[:, D + E + 2 : D + E + 6] = emotional_state
    comb_bf = comb.astype(BF16)

    w1 = np.ascontiguousarray(
        enc_w1.astype(BF16).reshape(DCH, 128, H1).transpose(1, 0, 2)
    )
    w2 = np.ascontiguousarray(
        enc_w2.astype(BF16).reshape(2, 128, E).transpose(1, 0, 2)
    )
    iw1p = np.zeros((TOTP, 64), dtype=f32)
    iw1p[:TOT] = imp_w1
    iw1 = np.ascontiguousarray(
        iw1p.astype(BF16).reshape(NCH, 128, 64).transpose(1, 0, 2)
    )
    iw2 = np.ascontiguousarray(imp_w2.astype(BF16).reshape(64, 1))
    b1 = np.ascontiguousarray(enc_b1.astype(f32).reshape(2, 128).T)
    b2 = np.ascontiguousarray(enc_b2.astype(f32).reshape(128, 1))
    ib1 = np.ascontiguousarray(imp_b1.astype(f32).reshape(64, 1))
    ib2 = np.full((128, 1), float(np.asarray(imp_b2).reshape(-1)[0]), dtype=f32)

    cn = np.linalg.norm(centroids.astype(f32), axis=1)
    centT = np.ascontiguousarray((centroids / cn[:, None]).T).astype(BF16)

    shared = dict(w1=w1, w2=w2, iw1=iw1, iw2=iw2, b1=b1, b2=b2, ib1=ib1,
                  ib2=ib2, centT=centT)
    in_maps = []
    for i in range(N_CORES):
        sl = slice(i * BL, (i + 1) * BL)
        m = dict(shared)
        m["combT"] = np.ascontiguousarray(comb_bf[sl].T)
        m["emo"] = np.ascontiguousarray(emotional_state[sl].astype(f32))
        in_maps.append(m)
    return in_maps


def kernel(cue, internal_state, reward, timestamp, emotional_state, centroids,
           enc_w1, enc_b1, enc_w2, enc_b2, imp_w1, imp_b1, imp_w2, imp_b2,
           top_k, **run_kwargs):
    assert int(top_k) == K, f"kernel hardcodes top_k={K}, got {top_k}"
    from concourse.bass_utils import run_bass_kernel_spmd

    if "nc" not in _CACHE:
        _CACHE["nc"] = _build_nc()
    nc = _CACHE["nc"]

    in_maps = _prep_inputs(
        np.asarray(cue, np.float32), np.asarray(internal_state, np.float32),
        np.asarray(reward, np.float32), np.asarray(timestamp, np.float32),
        np.asarray(emotional_state, np.float32),
        np.asarray(centroids, np.float32),
        np.asarray(enc_w1, np.float32), np.asarray(enc_b1, np.float32),
        np.asarray(enc_w2, np.float32), np.asarray(enc_b2, np.float32),
        np.asarray(imp_w1, np.float32), np.asarray(imp_b1, np.float32),
        np.asarray(imp_w2, np.float32), np.asarray(imp_b2, np.float32),
    )
    res = run_bass_kernel_spmd(
        nc, in_maps, core_ids=list(range(N_CORES)), **run_kwargs
    )
    out = np.concatenate([res.results[i]["out"] for i in range(N_CORES)], axis=0)
    _CACHE["last_results"] = res
    return out


# revision 7
# speedup vs baseline: 1.2591x; 1.2591x over previous
"""Trainium2 Bass kernel for the DifferentiableMemory scatter_memory problem.

Data-parallel over 8 NeuronCores: batch B=32768 is sharded into 8 x 4096 rows.
Host side does layout only (transpose/cast/concat/weight repack); all NN math
(encoder MLP, cosine sims, top-k, importance net) runs on device in bf16 with
fp32 PSUM accumulation.

Device dataflow (per core, 8 superblocks of 512 batch columns):
  activations live transposed [feature, batch]:
    xT        [128, 8, 512]  combined.T chunks (cue | istate | reward/ts/emo | pad)
    h1T       = gelu(W1.T @ xT + b1)        -> [256, 512] bf16
    encT      = W2.T @ h1T + b2             -> [128, 512] bf16
    ssq[b]    = ones.T @ (encT^2)           -> per-batch ||enc||^2 via PE
    sims[b,n] = (encT_q).T @ centT_scaled   -> [128, 500] fp32 (centT pre-divided
                 by ||c||; divide by ||enc|| AFTER top-8: positive per-row scale
                 preserves order). eps-clamp of the reference never binds here
                 (||enc||*||c|| >> 1e-8).
    top8      = nc.vector.max (one DVE instruction, sorted desc) -> take 5
    impT      = sigmoid(w2i.T @ gelu(W1i.T @ xT + b1i) + b2i) * mean(emo)
"""

import numpy as np
import ml_dtypes

BF16 = ml_dtypes.bfloat16

N_CORES = 8
B = 32768
BL = B // N_CORES          # 4096 rows per core
SB = 512                   # superblock: batch columns per iteration
NSB = BL // SB             # 8 superblocks
Q = SB // 128              # 4 x 128-row tiles per superblock
D = 768
H1 = 256
E = 128
N = 500
K = 5
TOT = 902
TOTP = 1024                # padded combined dim: 8 chunks of 128
NCH = TOTP // 128          # 8
DCH = D // 128             # 6

_CACHE = {}


def _build_nc():
    import concourse.bacc as bacc
    import concourse.bass as bass
    import concourse.tile as tile
    from concourse import mybir

    f32 = mybir.dt.float32
    bf16 = mybir.dt.bfloat16
    AF = mybir.ActivationFunctionType
    ts = bass.ts

    nc = bacc.Bacc(None, target_bir_lowering=False)

    combT = nc.dram_tensor("combT", [TOTP, BL], bf16, kind="ExternalInput")
    emo = nc.dram_tensor("emo", [BL, 4], f32, kind="ExternalInput")
    w1 = nc.dram_tensor("w1", [128, DCH, H1], bf16, kind="ExternalInput")
    w2 = nc.dram_tensor("w2", [128, 2, E], bf16, kind="ExternalInput")
    iw1 = nc.dram_tensor("iw1", [128, NCH, 64], bf16, kind="ExternalInput")
    iw2 = nc.dram_tensor("iw2", [64, 1], bf16, kind="ExternalInput")
    b1 = nc.dram_tensor("b1", [128, 2], f32, kind="ExternalInput")
    b2 = nc.dram_tensor("b2", [128, 1], f32, kind="ExternalInput")
    ib1 = nc.dram_tensor("ib1", [64, 1], f32, kind="ExternalInput")
    ib2 = nc.dram_tensor("ib2", [128, 1], f32, kind="ExternalInput")
    centT = nc.dram_tensor("centT", [128, N], bf16, kind="ExternalInput")
    out = nc.dram_tensor("out", [BL, K + 1], f32, kind="ExternalOutput")

    combT_r = combT.rearrange("(c p) b -> p c b", p=128)     # [128, 8, BL]
    emo_r = emo.rearrange("(x p) e -> p x e", p=128)         # [128, 32, 4]
    out_r = out.rearrange("(x p) j -> p x j", p=128)         # [128, 32, 6]

    with tile.TileContext(nc) as tc:
        with (
            tc.tile_pool(name="const", bufs=1) as cpool,
            tc.tile_pool(name="work", bufs=2) as wpool,
            tc.tile_pool(name="acc", bufs=1) as apool,
            tc.tile_pool(name="small", bufs=2) as opool,
            tc.tile_pool(name="psA", bufs=3, space="PSUM") as psA,
            tc.tile_pool(name="psS", bufs=3, space="PSUM") as psS,
            tc.tile_pool(name="psT", bufs=1, space="PSUM") as psT,
        ):
            # ---- constants / weights (loaded once) ----
            w1t = cpool.tile([128, DCH, H1], bf16)
            nc.sync.dma_start(w1t[:], w1[:])
            w2t = cpool.tile([128, 2, E], bf16)
            nc.sync.dma_start(w2t[:], w2[:])
            iw1t = cpool.tile([128, NCH, 64], bf16)
            nc.sync.dma_start(iw1t[:], iw1[:])
            iw2t = cpool.tile([64, 1], bf16)
            nc.sync.dma_start(iw2t[:], iw2[:])
            b1t = cpool.tile([128, 2], f32)
            nc.sync.dma_start(b1t[:], b1[:])
            b2t = cpool.tile([128, 1], f32)
            nc.sync.dma_start(b2t[:], b2[:])
            ib1t = cpool.tile([64, 1], f32)
            nc.sync.dma_start(ib1t[:], ib1[:])
            ib2t = cpool.tile([128, 1], f32)
            nc.sync.dma_start(ib2t[:], ib2[:])
            centTt = cpool.tile([128, N], bf16)
            nc.sync.dma_start(centTt[:], centT[:])
            onesE = cpool.tile([128, 1], bf16)
            nc.vector.memset(onesE[:], 1.0)

            # per-kernel accumulators (one column group per superblock);
            # sqrt/sigmoid/output assembly deferred to a single epilogue so
            # the ACT engine keeps the Gelu table resident all main loop.
            XT = NSB * Q  # 32 tiles of 128 rows
            ssq_all = apool.tile([128, XT], f32)
            ic_all = apool.tile([128, XT], f32)
            esum_all = apool.tile([128, XT], f32)
            top8_all = apool.tile([128, XT, 8], f32)

            for sb in range(NSB):
                # ---- load inputs for this superblock ----
                xt = wpool.tile([128, NCH, SB], bf16, tag="xt")
                nc.sync.dma_start(xt[:], combT_r[:, :, ts(sb, SB)])
                emot = wpool.tile([128, Q, 4], f32, tag="emot")
                nc.sync.dma_start(emot[:], emo_r[:, ts(sb, Q), :])

                # ---- encoder layer 1: h1T = gelu(W1.T @ xT + b1) ----
                h1 = wpool.tile([128, 2, SB], bf16, tag="h1")
                for half in range(2):
                    ps = psA.tile([128, SB], f32, tag="mm")
                    for c in range(DCH):
                        nc.tensor.matmul(
                            ps[:],
                            lhsT=w1t[:, c, ts(half, 128)],
                            rhs=xt[:, c, :],
                            start=(c == 0),
                            stop=(c == DCH - 1),
                        )
                    nc.scalar.activation(
                        h1[:, half, :], ps[:], AF.Gelu, bias=b1t[:, half : half + 1]
                    )

                # ---- encoder layer 2: encT = W2.T @ h1T + b2 ----
                ps_enc = psA.tile([128, SB], f32, tag="mm")
                for c in range(2):
                    nc.tensor.matmul(
                        ps_enc[:],
                        lhsT=w2t[:, c, :],
                        rhs=h1[:, c, :],
                        start=(c == 0),
                        stop=(c == 1),
                    )
                encb = wpool.tile([128, SB], bf16, tag="encb")
                nc.vector.tensor_scalar_add(encb[:], ps_enc[:], b2t[:])
                enc2 = wpool.tile([128, SB], bf16, tag="enc2")
                nc.vector.tensor_mul(enc2[:], encb[:], encb[:])

                # ---- ||enc||^2 per batch col via PE (partition reduction) ----
                ps_ssq = psT.tile([128, Q], f32, tag="tiny")
                for q in range(Q):
                    nc.tensor.matmul(
                        ps_ssq[:, q : q + 1],
                        lhsT=enc2[:, ts(q, 128)],
                        rhs=onesE[:],
                        start=True,
                        stop=True,
                    )
                nc.vector.tensor_copy(ssq_all[:, ts(sb, Q)], ps_ssq[:])

                # ---- sims + top8 per 128-row tile (max8 reads PSUM) ----
                for q in range(Q):
                    ps_sims = psS.tile([128, N], f32, tag="sims")
                    nc.tensor.matmul(
                        ps_sims[:],
                        lhsT=encb[:, ts(q, 128)],
                        rhs=centTt[:],
                        start=True,
                        stop=True,
                    )
                    nc.vector.max(top8_all[:, sb * Q + q, :], ps_sims[:])

                # ---- importance net ----
                ps_imp = psA.tile([64, SB], f32, tag="mm")
                for c in range(NCH):
                    nc.tensor.matmul(
                        ps_imp[:],
                        lhsT=iw1t[:, c, :],
                        rhs=xt[:, c, :],
                        start=(c == 0),
                        stop=(c == NCH - 1),
                    )
                himp = wpool.tile([64, SB], bf16, tag="himp")
                nc.scalar.activation(himp[:], ps_imp[:], AF.Gelu, bias=ib1t[:])

                ps_ic = psT.tile([128, Q], f32, tag="tiny")
                for q in range(Q):
                    nc.tensor.matmul(
                        ps_ic[:, q : q + 1],
                        lhsT=himp[:, ts(q, 128)],
                        rhs=iw2t[:],
                        start=True,
                        stop=True,
                    )
                nc.vector.tensor_copy(ic_all[:, ts(sb, Q)], ps_ic[:])
                nc.vector.reduce_sum(
                    esum_all[:, ts(sb, Q)], emot[:], axis=mybir.AxisListType.X
                )

            # ---- epilogue: one table load each for Sqrt / Sigmoid ----
            e_all = opool.tile([128, XT], f32, tag="e_all")
            nc.scalar.activation(e_all[:], ssq_all[:], AF.Sqrt)
            rinv_all = opool.tile([128, XT], f32, tag="rinv")
            nc.vector.reciprocal(rinv_all[:], e_all[:])
            sg_all = opool.tile([128, XT], f32, tag="sg")
            nc.scalar.activation(sg_all[:], ic_all[:], AF.Sigmoid, bias=ib2t[:])

            # top5 * (1/||enc||): broadcast rinv along the 8-wide top8 axis
            # via a step-0 free dim, then pack [.., 0:5] into the out tile.
            t8s = opool.tile([128, XT, 8], f32, tag="t8s")
            rinv_b = rinv_all[:].broadcast_to([128, XT, 8])
            nc.vector.tensor_mul(t8s[:], top8_all[:], rinv_b)

            ot = opool.tile([128, XT, K + 1], f32, tag="ot")
            nc.vector.tensor_copy(ot[:, :, 0:K], t8s[:, :, 0:K])
            # imp = sigmoid * 0.25 * sum(emo)
            nc.vector.scalar_tensor_tensor(
                ot[:, :, K],
                in0=sg_all[:],
                scalar=0.25,
                in1=esum_all[:],
                op0=mybir.AluOpType.mult,
                op1=mybir.AluOpType.mult,
            )
            nc.sync.dma_start(out_r[:], ot[:])

    nc.compile()
    return nc


def _prep_inputs(cue, internal_state, reward, timestamp, emotional_state,
                 centroids, enc_w1, enc_b1, enc_w2, enc_b2,
                 imp_w1, imp_b1, imp_w2, imp_b2):
    f32 = np.float32

    comb = np.zeros((B, TOTP), dtype=f32)
    comb[:, :D] = cue
    comb[:, D : D + E] = internal_state
    comb[:, D + E] = reward[:, 0]
    comb[:, D + E + 1] = timestamp[:, 0]
    comb[:, D + E + 2 : D + E + 6] = emotional_state
    comb_bf = comb.astype(BF16)

    w1 = np.ascontiguousarray(
        enc_w1.astype(BF16).reshape(DCH, 128, H1).transpose(1, 0, 2)
    )
    w2 = np.ascontiguousarray(
        enc_w2.astype(BF16).reshape(2, 128, E).transpose(1, 0, 2)
    )
    iw1p = np.zeros((TOTP, 64), dtype=f32)
    iw1p[:TOT] = imp_w1
    iw1 = np.ascontiguousarray(
        iw1p.astype(BF16).reshape(NCH, 128, 64).transpose(1, 0, 2)
    )
    iw2 = np.ascontiguousarray(imp_w2.astype(BF16).reshape(64, 1))
    b1 = np.ascontiguousarray(enc_b1.astype(f32).reshape(2, 128).T)
    b2 = np.ascontiguousarray(enc_b2.astype(f32).reshape(128, 1))
    ib1 = np.ascontiguousarray(imp_b1.astype(f32).reshape(64, 1))
    ib2 = np.full((128, 1), float(np.asarray(imp_b2).reshape(-1)[0]), dtype=f32)

    cn = np.linalg.norm(centroids.astype(f32), axis=1)
    centT = np.ascontiguousarray((centroids / cn[:, None]).T).astype(BF16)

    shared = dict(w1=w1, w2=w2, iw1=iw1, iw2=iw2, b1=b1, b2=b2, ib1=ib1,
                  ib2=ib2, centT=centT)
    in_maps = []
    for i in range(N_CORES):
        sl = slice(i * BL, (i + 1) * BL)
        m = dict(shared)
        m["combT"] = np.ascontiguousarray(comb_bf[sl].T)
        m["emo"] = np.ascontiguousarray(emotional_state[sl].astype(f32))
        in_maps.append(m)
    return in_maps


def kernel(cue, internal_state, reward, timestamp, emotional_state, centroids,
           enc_w1, enc_b1, enc_w2, enc_b2, imp_w1, imp_b1, imp_w2, imp_b2,
           top_k, **run_kwargs):
    assert int(top_k) == K, f"kernel hardcodes top_k={K}, got {top_k}"
    from concourse.bass_utils import run_bass_kernel_spmd

    if "nc" not in _CACHE:
        _CACHE["nc"] = _build_nc()
    nc = _CACHE["nc"]

    in_maps = _prep_inputs(
        np.asarray(cue, np.float32), np.asarray(internal_state, np.float32),
        np.asarray(reward, np.float32), np.asarray(timestamp, np.float32),
        np.asarray(emotional_state, np.float32),
        np.asarray(centroids, np.float32),
        np.asarray(enc_w1, np.float32), np.asarray(enc_b1, np.float32),
        np.asarray(enc_w2, np.float32), np.asarray(enc_b2, np.float32),
        np.asarray(imp_w1, np.float32), np.asarray(imp_b1, np.float32),
        np.asarray(imp_w2, np.float32), np.asarray(imp_b2, np.float32),
    )
    res = run_bass_kernel_spmd(
        nc, in_maps, core_ids=list(range(N_CORES)), **run_kwargs
    )
    out = np.concatenate([res.results[i]["out"] for i in range(N_CORES)], axis=0)
    _CACHE["last_results"] = res
    return out


# revision 15
# speedup vs baseline: 1.4737x; 1.1705x over previous
"""Trainium2 Bass kernel for the DifferentiableMemory scatter_memory problem.

Data-parallel over 8 NeuronCores: batch B=32768 is sharded into 8 x 4096 rows.
Host side does layout only (transpose/cast/concat/weight repack); all NN math
(encoder MLP, cosine sims, top-k, importance net) runs on device in bf16 with
fp32 PSUM accumulation.

Device dataflow (per core, 8 superblocks of 512 batch columns):
  activations live transposed [feature, batch]:
    xT        [128, 8, 512]  combined.T chunks (cue | istate | reward/ts/emo | pad)
    h1T       = gelu(W1.T @ xT + b1)        -> [256, 512] bf16
    encT      = W2.T @ h1T + b2             -> [128, 512] bf16
    ssq[b]    = ones.T @ (encT^2)           -> per-batch ||enc||^2 via PE
    sims[b,n] = (encT_q).T @ centT_scaled   -> [128, 500] fp32 (centT pre-divided
                 by ||c||; divide by ||enc|| AFTER top-8: positive per-row scale
                 preserves order). eps-clamp of the reference never binds here
                 (||enc||*||c|| >> 1e-8).
    top8      = nc.vector.max (one DVE instruction, sorted desc) -> take 5
    impT      = sigmoid(w2i.T @ gelu(W1i.T @ xT + b1i) + b2i) * mean(emo)
"""

import numpy as np
import ml_dtypes

BF16 = ml_dtypes.bfloat16

N_CORES = 8
B = 32768
BL = B // N_CORES          # 4096 rows per core
SB = 512                   # superblock: batch columns per iteration
NSB = BL // SB             # 8 superblocks
Q = SB // 128              # 4 x 128-row tiles per superblock
D = 768
H1 = 256
E = 128
N = 500
K = 5
TOT = 902
TOTP = 1024                # padded combined dim: 8 chunks of 128
NCH = TOTP // 128          # 8
DCH = D // 128             # 6

_CACHE = {}


def _build_nc():
    import concourse.bacc as bacc
    import concourse.bass as bass
    import concourse.tile as tile
    from concourse import mybir

    f32 = mybir.dt.float32
    bf16 = mybir.dt.bfloat16
    AF = mybir.ActivationFunctionType
    ts = bass.ts

    nc = bacc.Bacc(None, target_bir_lowering=False)

    combT = nc.dram_tensor("combT", [TOTP, BL], bf16, kind="ExternalInput")
    emo = nc.dram_tensor("emo", [128, BL // 128, 4], f32, kind="ExternalInput")
    w1 = nc.dram_tensor("w1", [128, DCH, H1], bf16, kind="ExternalInput")
    w2 = nc.dram_tensor("w2", [128, 2, E], bf16, kind="ExternalInput")
    iw1 = nc.dram_tensor("iw1", [128, NCH, 64], bf16, kind="ExternalInput")
    iw2 = nc.dram_tensor("iw2", [64, 1], bf16, kind="ExternalInput")
    b1 = nc.dram_tensor("b1", [128, 2], f32, kind="ExternalInput")
    b2 = nc.dram_tensor("b2", [128, 1], f32, kind="ExternalInput")
    ib1 = nc.dram_tensor("ib1", [64, 1], f32, kind="ExternalInput")
    ib2 = nc.dram_tensor("ib2", [128, 1], f32, kind="ExternalInput")
    centT = nc.dram_tensor("centT", [128, N], bf16, kind="ExternalInput")
    out = nc.dram_tensor("out", [128, (BL // 128) * (K + 1)], f32,
                         kind="ExternalOutput")

    combT_r = combT.rearrange("(c p) b -> p c b", p=128)     # [128, 8, BL]

    with tile.TileContext(nc) as tc:
        with (
            tc.tile_pool(name="const", bufs=1) as cpool,
            tc.tile_pool(name="work", bufs=2) as wpool,
            tc.tile_pool(name="acc", bufs=1) as apool,
            tc.tile_pool(name="small", bufs=2) as opool,
            tc.tile_pool(name="psA", bufs=3, space="PSUM") as psA,
            tc.tile_pool(name="psS", bufs=3, space="PSUM") as psS,
            tc.tile_pool(name="psT", bufs=1, space="PSUM") as psT,
        ):
            # ---- constants needed first (w1/b1 gate the first matmuls) ----
            w1t = cpool.tile([128, DCH, H1], bf16)
            nc.sync.dma_start(w1t[:], w1[:])
            b1t = cpool.tile([128, 2], f32)
            nc.sync.dma_start(b1t[:], b1[:])
            onesE = cpool.tile([128, 1], bf16)
            nc.vector.memset(onesE[:], 1.0)
            # remaining consts are DMA'd after sb0's x-tiles (see loop below)
            w2t = cpool.tile([128, 2, E], bf16)
            iw1t = cpool.tile([128, NCH, 64], bf16)
            iw2t = cpool.tile([64, 1], bf16)
            b2t = cpool.tile([128, 1], f32)
            ib1t = cpool.tile([64, 1], f32)
            ib2t = cpool.tile([128, 1], f32)
            centTt = cpool.tile([128, N], bf16)
            emot = cpool.tile([128, BL // 128, 4], f32)

            # per-kernel accumulators (one column group per superblock);
            # sqrt/sigmoid/output assembly deferred to a single epilogue so
            # the ACT engine keeps the Gelu table resident all main loop.
            XT = NSB * Q  # 32 tiles of 128 rows
            ssq_all = apool.tile([128, XT], f32)
            ic_all = apool.tile([128, XT], f32)
            esum_all = apool.tile([128, XT], f32)
            top8_all = apool.tile([128, XT, 8], f32)

            for sb in range(NSB):
                # ---- load inputs for this superblock, in 3 pieces so the
                # first matmuls start as soon as chunks 0-2 land ----
                xta = wpool.tile([128, 3, SB], bf16, tag="xta")
                nc.sync.dma_start(xta[:], combT_r[:, 0:3, ts(sb, SB)])
                xtb = wpool.tile([128, 3, SB], bf16, tag="xtb")
                nc.sync.dma_start(xtb[:], combT_r[:, 3:6, ts(sb, SB)])
                xtc = wpool.tile([128, 2, SB], bf16, tag="xtc")
                nc.sync.dma_start(xtc[:], combT_r[:, 6:8, ts(sb, SB)])
                if sb == 0:
                    # now queue the consts that are needed a bit later
                    nc.sync.dma_start(w2t[:], w2[:])
                    nc.sync.dma_start(centTt[:], centT[:])
                    nc.sync.dma_start(iw1t[:], iw1[:])
                    nc.sync.dma_start(iw2t[:], iw2[:])
                    nc.sync.dma_start(b2t[:], b2[:])
                    nc.sync.dma_start(ib1t[:], ib1[:])
                    nc.sync.dma_start(ib2t[:], ib2[:])
                    nc.sync.dma_start(emot[:], emo[:])

                def xchunk(c):
                    return (xta[:, c, :] if c < 3
                            else xtb[:, c - 3, :] if c < 6
                            else xtc[:, c - 6, :])

                # ---- encoder layer 1: h1T = gelu(W1.T @ xT + b1) ----
                h1 = wpool.tile([128, 2, SB], bf16, tag="h1")
                for half in range(2):
                    ps = psA.tile([128, SB], f32, tag="mm")
                    for c in range(DCH):
                        nc.tensor.matmul(
                            ps[:],
                            lhsT=w1t[:, c, ts(half, 128)],
                            rhs=xchunk(c),
                            start=(c == 0),
                            stop=(c == DCH - 1),
                        )
                    nc.scalar.activation(
                        h1[:, half, :], ps[:], AF.Gelu, bias=b1t[:, half : half + 1]
                    )

                # ---- encoder layer 2: encT = W2.T @ h1T + b2 ----
                ps_enc = psA.tile([128, SB], f32, tag="mm")
                for c in range(2):
                    nc.tensor.matmul(
                        ps_enc[:],
                        lhsT=w2t[:, c, :],
                        rhs=h1[:, c, :],
                        start=(c == 0),
                        stop=(c == 1),
                    )
                encb = wpool.tile([128, SB], bf16, tag="encb")
                nc.vector.tensor_scalar_add(encb[:], ps_enc[:], b2t[:])
                enc2 = wpool.tile([128, SB], bf16, tag="enc2")
                nc.vector.tensor_mul(enc2[:], encb[:], encb[:])

                # ---- ||enc||^2 per batch col via PE (partition reduction) ----
                ps_ssq = psT.tile([128, Q], f32, tag="tiny")
                for q in range(Q):
                    nc.tensor.matmul(
                        ps_ssq[:, q : q + 1],
                        lhsT=enc2[:, ts(q, 128)],
                        rhs=onesE[:],
                        start=True,
                        stop=True,
                    )
                nc.vector.tensor_copy(ssq_all[:, ts(sb, Q)], ps_ssq[:])

                # ---- sims + top8 per 128-row tile (max8 reads PSUM) ----
                for q in range(Q):
                    ps_sims = psS.tile([128, N], f32, tag="sims")
                    nc.tensor.matmul(
                        ps_sims[:],
                        lhsT=encb[:, ts(q, 128)],
                        rhs=centTt[:],
                        start=True,
                        stop=True,
                    )
                    nc.vector.max(top8_all[:, sb * Q + q, :], ps_sims[:])

                # ---- importance net ----
                ps_imp = psA.tile([64, SB], f32, tag="mm")
                for c in range(NCH):
                    nc.tensor.matmul(
                        ps_imp[:],
                        lhsT=iw1t[:, c, :],
                        rhs=xchunk(c),
                        start=(c == 0),
                        stop=(c == NCH - 1),
                    )
                himp = wpool.tile([64, SB], bf16, tag="himp")
                nc.scalar.activation(himp[:], ps_imp[:], AF.Gelu, bias=ib1t[:])

                ps_ic = psT.tile([128, Q], f32, tag="tiny")
                for q in range(Q):
                    nc.tensor.matmul(
                        ps_ic[:, q : q + 1],
                        lhsT=himp[:, ts(q, 128)],
                        rhs=iw2t[:],
                        start=True,
                        stop=True,
                    )
                nc.vector.tensor_copy(ic_all[:, ts(sb, Q)], ps_ic[:])

            # ---- epilogue: one table load each for Sqrt / Sigmoid ----
            nc.vector.reduce_sum(
                esum_all[:], emot[:], axis=mybir.AxisListType.X
            )
            e_all = opool.tile([128, XT], f32, tag="e_all")
            nc.scalar.activation(e_all[:], ssq_all[:], AF.Sqrt)
            rinv_all = opool.tile([128, XT], f32, tag="rinv")
            nc.vector.reciprocal(rinv_all[:], e_all[:])
            sg_all = opool.tile([128, XT], f32, tag="sg")
            nc.scalar.activation(sg_all[:], ic_all[:], AF.Sigmoid, bias=ib2t[:])

            # top5 * (1/||enc||): broadcast rinv along the 8-wide top8 axis
            # via a step-0 free dim, then pack [.., 0:5] into the out tile.
            t8s = opool.tile([128, XT, 8], f32, tag="t8s")
            rinv_b = rinv_all[:].broadcast_to([128, XT, 8])
            nc.vector.tensor_mul(t8s[:], top8_all[:], rinv_b)

            ot = opool.tile([128, XT, K + 1], f32, tag="ot")
            nc.vector.tensor_copy(ot[:, :, 0:K], t8s[:, :, 0:K])
            # imp = sigmoid * 0.25 * sum(emo)
            nc.vector.scalar_tensor_tensor(
                ot[:, :, K],
                in0=sg_all[:],
                scalar=0.25,
                in1=esum_all[:],
                op0=mybir.AluOpType.mult,
                op1=mybir.AluOpType.mult,
            )
            nc.sync.dma_start(out[:, :], ot[:])

    nc.compile()
    return nc


def _prep_inputs(cue, internal_state, reward, timestamp, emotional_state,
                 centroids, enc_w1, enc_b1, enc_w2, enc_b2,
                 imp_w1, imp_b1, imp_w2, imp_b2):
    f32 = np.float32

    comb = np.zeros((B, TOTP), dtype=f32)
    comb[:, :D] = cue
    comb[:, D : D + E] = internal_state
    comb[:, D + E] = reward[:, 0]
    comb[:, D + E + 1] = timestamp[:, 0]
    comb[:, D + E + 2 : D + E + 6] = emotional_state
    comb_bf = comb.astype(BF16)

    w1 = np.ascontiguousarray(
        enc_w1.astype(BF16).reshape(DCH, 128, H1).transpose(1, 0, 2)
    )
    w2 = np.ascontiguousarray(
        enc_w2.astype(BF16).reshape(2, 128, E).transpose(1, 0, 2)
    )
    iw1p = np.zeros((TOTP, 64), dtype=f32)
    iw1p[:TOT] = imp_w1
    iw1 = np.ascontiguousarray(
        iw1p.astype(BF16).reshape(NCH, 128, 64).transpose(1, 0, 2)
    )
    iw2 = np.ascontiguousarray(imp_w2.astype(BF16).reshape(64, 1))
    b1 = np.ascontiguousarray(enc_b1.astype(f32).reshape(2, 128).T)
    b2 = np.ascontiguousarray(enc_b2.astype(f32).reshape(128, 1))
    ib1 = np.ascontiguousarray(imp_b1.astype(f32).reshape(64, 1))
    ib2 = np.full((128, 1), float(np.asarray(imp_b2).reshape(-1)[0]), dtype=f32)

    cn = np.linalg.norm(centroids.astype(f32), axis=1)
    centT = np.ascontiguousarray((centroids / cn[:, None]).T).astype(BF16)

    shared = dict(w1=w1, w2=w2, iw1=iw1, iw2=iw2, b1=b1, b2=b2, ib1=ib1,
                  ib2=ib2, centT=centT)
    in_maps = []
    for i in range(N_CORES):
        sl = slice(i * BL, (i + 1) * BL)
        m = dict(shared)
        m["combT"] = np.ascontiguousarray(comb_bf[sl].T)
        # device-friendly emo layout: emo_dev[p, x, e] = emotional[x*128+p, e]
        m["emo"] = np.ascontiguousarray(
            emotional_state[sl].astype(f32).reshape(BL // 128, 128, 4)
            .transpose(1, 0, 2)
        )
        in_maps.append(m)
    return in_maps


def kernel(cue, internal_state, reward, timestamp, emotional_state, centroids,
           enc_w1, enc_b1, enc_w2, enc_b2, imp_w1, imp_b1, imp_w2, imp_b2,
           top_k, **run_kwargs):
    assert int(top_k) == K, f"kernel hardcodes top_k={K}, got {top_k}"
    from concourse.bass_utils import run_bass_kernel_spmd

    if "nc" not in _CACHE:
        _CACHE["nc"] = _build_nc()
    nc = _CACHE["nc"]

    in_maps = _prep_inputs(
        np.asarray(cue, np.float32), np.asarray(internal_state, np.float32),
        np.asarray(reward, np.float32), np.asarray(timestamp, np.float32),
        np.asarray(emotional_state, np.float32),
        np.asarray(centroids, np.float32),
        np.asarray(enc_w1, np.float32), np.asarray(enc_b1, np.float32),
        np.asarray(enc_w2, np.float32), np.asarray(enc_b2, np.float32),
        np.asarray(imp_w1, np.float32), np.asarray(imp_b1, np.float32),
        np.asarray(imp_w2, np.float32), np.asarray(imp_b2, np.float32),
    )
    res = run_bass_kernel_spmd(
        nc, in_maps, core_ids=list(range(N_CORES)), **run_kwargs
    )
    # device out is [128, XT*6] with out_dev[p, x*6+j] = out[x*128+p, j]
    parts = []
    for i in range(N_CORES):
        od = res.results[i]["out"].reshape(128, BL // 128, K + 1)
        parts.append(np.ascontiguousarray(od.transpose(1, 0, 2)).reshape(BL, K + 1))
    out = np.concatenate(parts, axis=0)
    _CACHE["last_results"] = res
    return out
